# revision 19
# baseline (speedup 1.0000x reference)
"""Trainium2 distributed kernel for nn_ACDE_87531433492502 (moe_routing).

Reference (N=65536 pixels, F=224 features, P=6 classes, H=256):
    h      = relu(Y @ W1 + b1); h = relu(h @ W2 + b2); logits = h @ W3 + b3
    cls    = argmax(abundance, -1)
    w      = per-class masked softmax of logits over the pixel axis
    M[p,f] = sum_n w[p,n,f] * Y[n,f]   (0 for empty classes)
    Y_hat  = abundance @ M

Distribution strategy (8 cores, data-parallel over pixels):
  * Pixels are sorted by class on the host and dealt round-robin so every
    core holds the same per-class segment structure [L_0..L_5] (pad slots get
    Y=0 / A=0). The per-class masked softmax sums then become contiguous
    free-axis segment reductions in the transposed (feature-major) layout —
    no on-device transposes or mask matmuls are needed.
  * MLP runs in transposed layout (features on partitions, pixels on the free
    axis) in bf16 with f32 PSUM accumulation, weight-stationary in groups of
    4 pixel chunks so LDWEIGHTS amortizes and PE stays dense.
  * Softmax shift: a per-feature LOCAL max is used for exp (safe: values <=1)
    so the max-AllReduce overlaps phase-2 compute; the local sums are then
    rescaled by exp(localmax - globalmax) before the sum-AllReduce. The
    softmax ratio sy/se is shift-invariant, so any consistent shift is exact.
    (b3 shifts logits per-feature uniformly over pixels, so it cancels in the
    softmax and is not applied on device.)
  * Pad slots flow through the MLP (Y=0 => they add k_p * E_pad to se and
    exactly 0 to sy); their se contribution is subtracted exactly using an
    actually-computed pad column of E and the host-known pad counts.
  * One AllReduce-max ([224] f32) + one AllReduce-add ([224,12] f32); the
    replicated M = sy/se feeds the data-parallel Y_hat^T = M^T A^T GEMM,
    which runs 4 pixel-quarters concurrently via PE row tiling (K=6 each).
"""

import sys
from math import ceil

for _p in ("/opt/trn_rl_repo", "/root/.axon_site/_ro/trn_rl_repo"):
    if _p not in sys.path:
        sys.path.insert(0, _p)

import numpy as np
import ml_dtypes

N_FULL = 65536
F = 224
P = 6
H = 256
N_CORES = 8
CH = 512   # pixels per MLP chunk
GRP = 4    # chunks per weight-stationary group
QUARTERS = 4  # final-GEMM row-tiling width

BF16 = ml_dtypes.bfloat16

USE_ACT_ACCUM = True   # se via exp's accum_out (verified on HW)
USE_STT_ACCUM = True   # sy via scalar_tensor_tensor's accum_out (to verify)


def _pieces(segs, n_loc):
    """(class, start, end) pieces: segments intersected with CH chunks."""
    out = []
    off = 0
    for p, L in enumerate(segs):
        a = off
        off += L
        while a < off:
            b = min(off, (a // CH + 1) * CH)
            out.append((p, a, b))
            a = b
    return out


def build(n_loc, segs, n_cores=N_CORES):
    import concourse.bass as bass  # noqa: F401
    import concourse.mybir as mybir
    from concourse import bacc
    from concourse import tile
    from contextlib import ExitStack

    dt = mybir.dt
    AF = mybir.ActivationFunctionType
    ALU = mybir.AluOpType

    nch = n_loc // CH
    n4 = n_loc // QUARTERS
    assert n4 % 128 == 0
    offs = [0]
    for L in segs:
        offs.append(offs[-1] + L)
    assert offs[-1] < n_loc  # at least one guaranteed tail pad column
    pieces = _pieces(segs, n_loc)
    pad_col = offs[-1]
    pad_ch, pad_off = pad_col // CH, pad_col % CH
    rg = [list(range(n_cores))]

    nc = bacc.Bacc(
        "TRN2", target_bir_lowering=False, debug=False, num_devices=n_cores
    )

    # ---- external I/O ----
    yt = nc.dram_tensor("yt", [F, n_loc], dt.bfloat16, kind="ExternalInput")
    # abundance^T packed for row-tiling: quarter q lives at partitions 32q..32q+5
    at4 = nc.dram_tensor("at4", [128, n4], dt.bfloat16, kind="ExternalInput")
    w1 = nc.dram_tensor("w1", [F, H], dt.bfloat16, kind="ExternalInput")
    w2 = nc.dram_tensor("w2", [H, H], dt.bfloat16, kind="ExternalInput")
    w3 = nc.dram_tensor("w3", [H, F], dt.bfloat16, kind="ExternalInput")
    b1 = nc.dram_tensor("b1", [H, 1], dt.float32, kind="ExternalInput")
    b2 = nc.dram_tensor("b2", [H, 1], dt.float32, kind="ExternalInput")
    kneg = nc.dram_tensor("kneg", [128, P], dt.float32, kind="ExternalInput")
    ident = nc.dram_tensor("ident", [128, 128], dt.bfloat16, kind="ExternalInput")
    out = nc.dram_tensor("out", [F, n_loc], dt.float32, kind="ExternalOutput")

    FT = ((128, 0), (96, 128))   # feature-partition tiles of 224
    HT = ((128, 0), (128, 128))  # hidden-partition tiles of 256

    with tile.TileContext(nc) as tc, ExitStack() as ctx:
        consts = ctx.enter_context(tc.tile_pool(name="consts", bufs=1))
        chp = ctx.enter_context(tc.tile_pool(name="chp", bufs=1))
        hp = ctx.enter_context(tc.tile_pool(name="hp", bufs=GRP))
        work = ctx.enter_context(tc.tile_pool(name="work", bufs=3))
        pp = ctx.enter_context(tc.tile_pool(name="pp", space="PSUM", bufs=8))
        dram = ctx.enter_context(tc.tile_pool(name="dram", bufs=1, space="DRAM"))

        # ---- collective bounce buffers ----
        mx_in = dram.tile([F, 1], dt.float32, name="mx_in")
        mx_out = dram.tile([F, 1], dt.float32, addr_space="Shared", name="mx_out")
        s_in = dram.tile([F, 2 * P], dt.float32, name="s_in")
        s_out = dram.tile([F, 2 * P], dt.float32, addr_space="Shared", name="s_out")

        # ---- constants ----
        def load_w(name, w_dram, tiles):
            sb = []
            for k, (ks, kofs) in enumerate(tiles):
                t = consts.tile([ks, w_dram.shape[1]], dt.bfloat16, name=f"{name}{k}")
                nc.sync.dma_start(out=t, in_=w_dram[kofs:kofs + ks, :])
                sb.append(t)
            return sb

        w1_sb = load_w("w1sb", w1, FT)
        w2_sb = load_w("w2sb", w2, HT)
        w3_sb = load_w("w3sb", w3, HT)
        b1_sb = []
        b2_sb = []
        for m, (ms, mofs) in enumerate(HT):
            t1 = consts.tile([ms, 1], dt.float32, name=f"b1sb{m}")
            nc.sync.dma_start(out=t1, in_=b1[mofs:mofs + ms, :])
            b1_sb.append(t1)
            t2 = consts.tile([ms, 1], dt.float32, name=f"b2sb{m}")
            nc.sync.dma_start(out=t2, in_=b2[mofs:mofs + ms, :])
            b2_sb.append(t2)
        kneg_sb = consts.tile([128, P], dt.float32, name="kneg_sb")
        nc.sync.dma_start(out=kneg_sb, in_=kneg[:, :])
        ident_sb = consts.tile([128, 128], dt.bfloat16, name="ident_sb")
        nc.sync.dma_start(out=ident_sb, in_=ident[:, :])
        at4_sb = consts.tile([128, n4], dt.bfloat16, name="at4_sb")
        nc.sync.dma_start(out=at4_sb, in_=at4[:, :])

        # ---- per-chunk persistent tensors ----
        yt_ch = [[None] * nch for _ in range(2)]
        lg_ch = [[None] * nch for _ in range(2)]
        et_ch = [[None] * nch for _ in range(2)]
        for c in range(nch):
            sl = slice(c * CH, (c + 1) * CH)
            for m, (ms, mofs) in enumerate(FT):
                t = chp.tile([ms, CH], dt.bfloat16, name=f"yt{m}_{c}",
                             tag=f"yt{m}_{c}")
                nc.sync.dma_start(out=t, in_=yt[mofs:mofs + ms, sl])
                yt_ch[m][c] = t

        # local per-feature max partials, one column per chunk
        lmaxp = [consts.tile([ms, nch], dt.float32, name=f"lmaxp{m}")
                 for m, (ms, _) in enumerate(FT)]

        # ================= phase 1: MLP into lg (weight-stationary groups) ====
        for g0 in range(0, nch, GRP):
            cs = range(g0, min(g0 + GRP, nch))

            h1 = {}
            for m, (ms, mofs) in enumerate(HT):
                ps1 = {}
                for ci in cs:
                    ps1[ci] = pp.tile([128, CH], dt.float32,
                                      name=f"ps1_{m}_{ci}", tag="ps")
                for k in range(2):
                    lhs = w1_sb[k][:, mofs:mofs + ms]
                    for ci in cs:
                        nc.tensor.matmul(ps1[ci], lhs, yt_ch[k][ci],
                                         start=(k == 0), stop=(k == 1))
                for ci in cs:
                    ht = hp.tile([128, CH], dt.bfloat16,
                                   name=f"h1_{m}_{ci}", tag=f"h1_{m}")
                    nc.scalar.activation(ht, ps1[ci], AF.Relu,
                                         bias=b1_sb[m], scale=1.0)
                    h1[m, ci] = ht

            h2 = {}
            for m, (ms, mofs) in enumerate(HT):
                ps2 = {}
                for ci in cs:
                    ps2[ci] = pp.tile([128, CH], dt.float32,
                                      name=f"ps2_{m}_{ci}", tag="ps")
                for k in range(2):
                    lhs = w2_sb[k][:, mofs:mofs + ms]
                    for ci in cs:
                        nc.tensor.matmul(ps2[ci], lhs, h1[k, ci],
                                         start=(k == 0), stop=(k == 1))
                for ci in cs:
                    ht = hp.tile([128, CH], dt.bfloat16,
                                   name=f"h2_{m}_{ci}", tag=f"h2_{m}")
                    nc.vector.tensor_scalar(ht, ps2[ci], b2_sb[m], 0.0,
                                            op0=ALU.add, op1=ALU.max)
                    h2[m, ci] = ht

            # L3: logits = W3^T @ h2 (b3 cancels in the softmax)
            for m, (ms, mofs) in enumerate(FT):
                ps3 = {}
                for ci in cs:
                    ps3[ci] = pp.tile([128, CH], dt.float32,
                                      name=f"ps3_{m}_{ci}", tag="ps")
                for k in range(2):
                    lhs = w3_sb[k][:, mofs:mofs + ms]
                    for ci in cs:
                        nc.tensor.matmul(ps3[ci][0:ms], lhs, h2[k, ci],
                                         start=(k == 0), stop=(k == 1))
                for ci in cs:
                    lg = chp.tile([ms, CH], dt.float32, name=f"lg{m}_{ci}",
                                  tag=f"lg{m}_{ci}")
                    nc.scalar.copy(out=lg, in_=ps3[ci][0:ms])
                    lg_ch[m][ci] = lg
                    # per-chunk local max partial straight from PSUM (DVE)
                    nc.vector.tensor_reduce(
                        out=lmaxp[m][:, ci:ci + 1], in_=ps3[ci][0:ms],
                        op=ALU.max, axis=mybir.AxisListType.X,
                    )

        # ======== local per-feature max; AllReduce-max runs overlapped ========
        lmax = []
        nlmax = []
        for m, (ms, mofs) in enumerate(FT):
            t = consts.tile([ms, 1], dt.float32, name=f"lmax{m}")
            nc.vector.tensor_reduce(out=t, in_=lmaxp[m], op=ALU.max,
                                    axis=mybir.AxisListType.X)
            lmax.append(t)
            ng = consts.tile([ms, 1], dt.float32, name=f"nlmax{m}")
            nc.vector.tensor_scalar(ng, t, -1.0, None, op0=ALU.mult)
            nlmax.append(ng)
            nc.gpsimd.dma_start(out=mx_in[mofs:mofs + ms, :], in_=t)

        nc.gpsimd.collective_compute(
            "AllReduce", ALU.max, replica_groups=rg,
            ins=[mx_in.opt()], outs=[mx_out.opt()],
        )

        # ============ phase 2: E = exp(lg - lmax), piecewise segment sums =====
        npc = len(pieces)
        seP = [consts.tile([ms, npc], dt.float32, name=f"seP{m}")
               for m, (ms, _) in enumerate(FT)]
        syP = [consts.tile([ms, npc], dt.float32, name=f"syP{m}")
               for m, (ms, _) in enumerate(FT)]

        for m, (ms, mofs) in enumerate(FT):
            for c in range(nch):
                et = chp.tile([ms, CH], dt.bfloat16, name=f"et{m}_{c}",
                              tag=f"et{m}_{c}")
                et_ch[m][c] = et
            for j, (p, a, b) in enumerate(pieces):
                c = a // CH
                sl = slice(a - c * CH, b - c * CH)
                if USE_ACT_ACCUM:
                    nc.scalar.activation(
                        out=et_ch[m][c][:, sl], in_=lg_ch[m][c][:, sl],
                        func=AF.Exp, bias=nlmax[m], scale=1.0,
                        accum_out=seP[m][:, j:j + 1],
                    )
                else:
                    nc.scalar.activation(
                        out=et_ch[m][c][:, sl], in_=lg_ch[m][c][:, sl],
                        func=AF.Exp, bias=nlmax[m], scale=1.0,
                    )
                    nc.vector.tensor_reduce(
                        out=seP[m][:, j:j + 1], in_=et_ch[m][c][:, sl],
                        op=ALU.add, axis=mybir.AxisListType.X,
                    )
                prod = work.tile([ms, b - a], dt.bfloat16,
                                 name=f"prod{m}_{j}", tag=f"prod{m}")
                if USE_STT_ACCUM:
                    nc.vector.scalar_tensor_tensor(
                        out=prod, in0=et_ch[m][c][:, sl], scalar=1.0,
                        in1=yt_ch[m][c][:, sl], op0=ALU.mult, op1=ALU.mult,
                        accum_out=syP[m][:, j:j + 1],
                    )
                else:
                    nc.vector.tensor_tensor(out=prod, in0=et_ch[m][c][:, sl],
                                            in1=yt_ch[m][c][:, sl], op=ALU.mult)
                    nc.vector.tensor_reduce(
                        out=syP[m][:, j:j + 1], in_=prod,
                        op=ALU.add, axis=mybir.AxisListType.X,
                    )
            # guaranteed pad column right after the last segment
            nc.scalar.activation(
                out=et_ch[m][pad_ch][:, pad_off:pad_off + 1],
                in_=lg_ch[m][pad_ch][:, pad_off:pad_off + 1],
                func=AF.Exp, bias=nlmax[m], scale=1.0,
            )

        # combine piece partials into per-class sums
        se_t = [consts.tile([ms, P], dt.float32, name=f"se_t{m}")
                for m, (ms, _) in enumerate(FT)]
        sy_t = [consts.tile([ms, P], dt.float32, name=f"sy_t{m}")
                for m, (ms, _) in enumerate(FT)]
        for m, (ms, mofs) in enumerate(FT):
            nc.vector.memset(se_t[m], 0.0)
            nc.vector.memset(sy_t[m], 0.0)
            for p in range(P):
                js = [j for j, pc in enumerate(pieces) if pc[0] == p]
                if not js:
                    continue
                j0, j1 = js[0], js[-1] + 1
                nc.vector.tensor_reduce(
                    out=se_t[m][:, p:p + 1], in_=seP[m][:, j0:j1],
                    op=ALU.add, axis=mybir.AxisListType.X,
                )
                nc.vector.tensor_reduce(
                    out=sy_t[m][:, p:p + 1], in_=syP[m][:, j0:j1],
                    op=ALU.add, axis=mybir.AxisListType.X,
                )

        # pad correction: se -= k_p * E_pad  (exact, E_pad actually computed)
        se_c = [consts.tile([ms, P], dt.float32, name=f"se_c{m}")
                for m, (ms, _) in enumerate(FT)]
        for m, (ms, mofs) in enumerate(FT):
            ep32 = consts.tile([ms, 1], dt.float32, name=f"ep32_{m}")
            nc.vector.tensor_copy(
                out=ep32, in_=et_ch[m][pad_ch][:, pad_off:pad_off + 1])
            nc.vector.scalar_tensor_tensor(
                out=se_c[m], in0=kneg_sb[0:ms, :], scalar=ep32, in1=se_t[m],
                op0=ALU.mult, op1=ALU.add,
            )

        # global max arrives; rescale local sums by exp(lmax - gmax)
        se_s = [consts.tile([ms, P], dt.float32, name=f"se_s{m}")
                for m, (ms, _) in enumerate(FT)]
        sy_s = [consts.tile([ms, P], dt.float32, name=f"sy_s{m}")
                for m, (ms, _) in enumerate(FT)]
        for m, (ms, mofs) in enumerate(FT):
            g = consts.tile([ms, 1], dt.float32, name=f"gmax{m}")
            nc.gpsimd.dma_start(out=g, in_=mx_out[mofs:mofs + ms, :])
            r = consts.tile([ms, 1], dt.float32, name=f"rfac{m}")
            nc.scalar.activation(out=r, in_=g, func=AF.Exp,
                                 bias=lmax[m], scale=-1.0)
            nc.vector.tensor_scalar(se_s[m], se_c[m], r, None, op0=ALU.mult)
            nc.vector.tensor_scalar(sy_s[m], sy_t[m], r, None, op0=ALU.mult)
            nc.sync.dma_start(out=s_in[mofs:mofs + ms, 0:P], in_=se_s[m])
            nc.sync.dma_start(out=s_in[mofs:mofs + ms, P:2 * P], in_=sy_s[m])

        # ================= AllReduce #2: global se / sy =================
        nc.gpsimd.collective_compute(
            "AllReduce", ALU.add, replica_groups=rg,
            ins=[s_in.opt()], outs=[s_out.opt()],
        )

        # M^T = sy/se in feature-major layout, then PE-transpose to [P, F]
        m_ps = pp.tile([P, F], dt.bfloat16, name="m_ps", tag="ps")
        for m, (ms, mofs) in enumerate(FT):
            srt = consts.tile([ms, 2 * P], dt.float32, name=f"srt{m}")
            nc.sync.dma_start(out=srt, in_=s_out[mofs:mofs + ms, :])
            rec = consts.tile([ms, P], dt.float32, name=f"rec{m}")
            nc.vector.tensor_scalar(rec, srt[:, 0:P], 1e-30, None, op0=ALU.max)
            nc.vector.reciprocal(out=rec, in_=rec)
            mt = consts.tile([ms, P], dt.bfloat16, name=f"mt{m}")
            nc.vector.tensor_tensor(out=mt, in0=srt[:, P:2 * P], in1=rec,
                                    op=ALU.mult)
            nc.tensor.transpose(m_ps[:, mofs:mofs + ms], mt, ident_sb[0:ms, 0:ms])
        # replicate M at partition offsets 0/32/64/96 for row-tiled GEMM
        m_sb4 = consts.tile([128, F], dt.bfloat16, name="m_sb4")
        for q in range(QUARTERS):
            nc.vector.tensor_copy(out=m_sb4[32 * q:32 * q + P, :], in_=m_ps)

        # ==== phase 3: Y_hat^T = M^T @ A^T, 4 pixel-quarters concurrently ====
        n_j = ceil(n4 / CH)
        for jc in range(n_j):
            j0 = jc * CH
            jw = min(CH, n4 - j0)
            for m, (ms, mofs) in enumerate(FT):
                pss = []
                for q in range(QUARTERS):
                    ps = pp.tile([128, CH], dt.float32,
                                 name=f"ops_{m}_{jc}_{q}", tag="ps")
                    nc.tensor.matmul(
                        ps[0:ms, 0:jw],
                        m_sb4[32 * q:32 * q + P, mofs:mofs + ms],
                        at4_sb[32 * q:32 * q + P, j0:j0 + jw],
                        start=True, stop=True,
                        tile_position=(32 * q, 0),
                    )
                    pss.append(ps)
                for q in range(QUARTERS):
                    osb = work.tile([ms, jw], dt.float32,
                                    name=f"osb_{m}_{jc}_{q}", tag=f"osb{m}")
                    if (q + m) % 2 == 0:
                        nc.scalar.copy(out=osb, in_=pss[q][0:ms, 0:jw])
                    else:
                        nc.vector.tensor_copy(out=osb, in_=pss[q][0:ms, 0:jw])
                    nc.sync.dma_start(
                        out=out[mofs:mofs + ms, q * n4 + j0:q * n4 + j0 + jw],
                        in_=osb)

    nc.compile()
    return nc


_CACHE = {}


def _get_nc(n_loc, segs):
    key = (n_loc, tuple(segs))
    if key not in _CACHE:
        _CACHE[key] = build(n_loc, tuple(segs))
    return _CACHE[key]


def prepare(inputs, n_cores=N_CORES):
    """Class-sort + balance-deal pixels; build per-core inputs and metadata."""
    A = np.asarray(inputs["abundance_matrix"], dtype=np.float32)
    Y = np.asarray(inputs["Y"], dtype=np.float32)
    n_full = A.shape[0]

    cls = np.argmax(A, axis=1)
    idx_p = [np.flatnonzero(cls == p) for p in range(P)]
    segs = [len(ix[0::n_cores]) for ix in idx_p]  # ceil(count/n_cores)
    n_loc = (CH * QUARTERS) * ceil((sum(segs) + 1) / (CH * QUARTERS))
    offs = np.concatenate(([0], np.cumsum(segs)))

    src = np.full((n_cores, n_loc), -1, dtype=np.int64)
    kcore = np.zeros((n_cores, P), np.float32)
    for p in range(P):
        for i in range(n_cores):
            mine = idx_p[p][i::n_cores]
            src[i, offs[p]:offs[p] + len(mine)] = mine
            kcore[i, p] = segs[p] - len(mine)

    W1 = np.asarray(inputs["W1"], dtype=np.float32).astype(BF16)
    W2 = np.asarray(inputs["W2"], dtype=np.float32).astype(BF16)
    W3 = np.asarray(inputs["W3"], dtype=np.float32).astype(BF16)
    b1 = np.asarray(inputs["b1"], dtype=np.float32).reshape(-1, 1)
    b2 = np.asarray(inputs["b2"], dtype=np.float32).reshape(-1, 1)
    ident = np.eye(128, dtype=BF16)
    n4 = n_loc // QUARTERS

    in_maps = []
    for i in range(n_cores):
        sel = src[i]
        valid = sel >= 0
        Yl = np.zeros((n_loc, Y.shape[1]), np.float32)
        Yl[valid] = Y[sel[valid]]
        Al = np.zeros((n_loc, P), np.float32)
        Al[valid] = A[sel[valid]]
        atT = Al.T.astype(BF16)  # [P, n_loc]
        at4 = np.zeros((128, n4), BF16)
        for q in range(QUARTERS):
            at4[32 * q:32 * q + P, :] = atT[:, q * n4:(q + 1) * n4]
        in_maps.append({
            "yt": np.ascontiguousarray(Yl.T).astype(BF16),
            "at4": at4,
            "w1": W1, "w2": W2, "w3": W3, "b1": b1, "b2": b2,
            "kneg": np.tile(-kcore[i], (128, 1)).astype(np.float32),
            "ident": ident,
        })
    return in_maps, src, segs, n_loc, n_full


def kernel(**inputs):
    from concourse.bass_utils import run_bass_kernel_spmd

    in_maps, src, segs, n_loc, n_full = prepare(inputs)
    nc = _get_nc(n_loc, segs)
    res = run_bass_kernel_spmd(nc, in_maps, core_ids=list(range(N_CORES)))
    out = np.empty((n_full, F), dtype=np.float32)
    for i in range(N_CORES):
        sel = src[i]
        valid = sel >= 0
        out[sel[valid]] = res.results[i]["out"].T[valid]
    return out


if __name__ == "__main__":
    import importlib.util

    spec = importlib.util.spec_from_file_location("reference", "/root/problem/reference.py")
    ref = importlib.util.module_from_spec(spec)
    spec.loader.exec_module(ref)
    inputs = {k: np.asarray(v) for k, v in ref.setup_inputs().items()}
    got = kernel(**inputs)
    exp = np.asarray(ref.reference(**inputs))
    rel = np.linalg.norm(got - exp) / np.linalg.norm(exp)
    print("Relative error:", rel)
